# revision 24
# baseline (speedup 1.0000x reference)
import sys

sys.path.insert(0, '/opt/trn_rl_repo')

import numpy as np

import concourse.bacc as bacc
import concourse.bass as bass
import concourse.mybir as mybir
import concourse.tile as tile
from concourse.bass_utils import run_bass_kernel_spmd

N = 8
C = 256
CQ = 64
P = 4096
PCH = 512
NCH = P // PCH      # 8 p-chunks
NQT = P // 128      # 32 q-tiles
F32 = mybir.dt.float32
F32R = mybir.dt.float32r
BF16 = mybir.dt.bfloat16


def _dedup_ldweights(nc):
    # drop InstLdweights that reload the PE array with the weights it already
    # holds; their waits/updates move onto the following matmul (legalized by
    # the subsequent generate_event_semaphores pass)
    PE = mybir.EngineType.PE
    for blkw in nc.bb_map.values():
        insts = blkw.bb.instructions
        cur = None
        drop = []
        for idx, i in enumerate(insts):
            if getattr(i, 'engine', None) != PE:
                continue
            tn = type(i).__name__
            if tn == 'InstLdweights':
                key = (str(i.ins[0]), str(getattr(i, 'tile_position', None)))
                if cur is not None and key == cur:
                    drop.append(idx)
                else:
                    cur = key
            elif tn == 'InstMatmult':
                if i.ldweights is not False:
                    cur = (str(i.ins[1]),
                           str(getattr(i, 'tile_position', None)))
            elif tn == 'InstEventSemaphore':
                pass
            else:
                cur = None
        for idx in reversed(drop):
            ld = insts[idx]
            mm = None
            for j in range(idx + 1, len(insts)):
                if type(insts[j]).__name__ == 'InstMatmult':
                    mm = insts[j]
                    break
            assert mm is not None
            si = ld.sync_info
            if si is not None:
                msi = mm.sync_info
                if msi is None:
                    mm.sync_info = si
                else:
                    msi.on_wait.extend(si.on_wait)
                    msi.on_update.extend(si.on_update)
            del insts[idx]
        if drop:
            left = sum(1 for i in insts
                       if type(i).__name__ == 'InstLdweights')
            assert left + len(drop) >= len(drop)


def build_bass():
    nc = bacc.Bacc(None, target_bir_lowering=False)
    x_d = nc.dram_tensor("x", [C, P], F32, kind="ExternalInput")
    wqt_d = nc.dram_tensor("wqt", [C, CQ], F32, kind="ExternalInput")
    wkt_d = nc.dram_tensor("wkt", [C, CQ], F32, kind="ExternalInput")
    wvt_d = nc.dram_tensor("wvt", [C, C], F32, kind="ExternalInput")
    bq_d = nc.dram_tensor("bq", [CQ, 1], F32, kind="ExternalInput")
    bk_d = nc.dram_tensor("bk", [CQ, 1], F32, kind="ExternalInput")
    bvb_d = nc.dram_tensor("bvb", [128, C], F32, kind="ExternalInput")
    ones_d = nc.dram_tensor("ones", [128, 128], F32, kind="ExternalInput")
    g_d = nc.dram_tensor("gamma", [1, 1], F32, kind="ExternalInput")
    out_d = nc.dram_tensor("out", [C, P], F32, kind="ExternalOutput")

    AF = mybir.ActivationFunctionType

    with tile.TileContext(nc) as tc:
        with (
            tc.tile_pool(name="big", bufs=1) as big,
            tc.tile_pool(name="ework", bufs=3) as ework,
            tc.tile_pool(name="small", bufs=2) as small,
            tc.tile_pool(name="outp", bufs=2) as outp,
        ):
            x0 = big.tile([128, P], F32R)
            x1 = big.tile([128, P], F32R)
            # q/k duplicated on partitions 64-127 so the St matmul pair can
            # run on both PE row-quadrants concurrently (tile_position)
            qsb = big.tile([128, P], BF16)
            ksb = big.tile([128, P], BF16)
            vt = big.tile([128, NQT * C], BF16)
            esum0 = big.tile([128, PCH], F32R)
            esum1 = big.tile([128, PCH], F32R)
            wqt0 = big.tile([128, CQ], F32R)
            wqt1 = big.tile([128, CQ], F32R)
            wkt0 = big.tile([128, CQ], F32R)
            wkt1 = big.tile([128, CQ], F32R)
            wvt0 = big.tile([128, C], F32R)
            wvt1 = big.tile([128, C], F32R)
            bq_sb = big.tile([CQ, 1], F32)
            bk_sb = big.tile([CQ, 1], F32)
            bvb_sb = big.tile([128, C], F32)
            g_sb = big.tile([1, 1], F32)
            ones_col = big.tile([128, 1], F32R)
            ones_row = big.tile([1, 128], F32R)

            nc.gpsimd.dma_start(x0[:], x_d[0:128, :].bitcast(F32R))
            nc.gpsimd.dma_start(x1[:], x_d[128:256, :].bitcast(F32R))
            nc.gpsimd.dma_start(wqt0[:], wqt_d[0:128, :].bitcast(F32R))
            nc.gpsimd.dma_start(wqt1[:], wqt_d[128:256, :].bitcast(F32R))
            nc.gpsimd.dma_start(wkt0[:], wkt_d[0:128, :].bitcast(F32R))
            nc.gpsimd.dma_start(wkt1[:], wkt_d[128:256, :].bitcast(F32R))
            nc.gpsimd.dma_start(wvt0[:], wvt_d[0:128, :].bitcast(F32R))
            nc.gpsimd.dma_start(wvt1[:], wvt_d[128:256, :].bitcast(F32R))
            nc.gpsimd.dma_start(bq_sb[:], bq_d[:])
            nc.gpsimd.dma_start(bk_sb[:], bk_d[:])
            nc.gpsimd.dma_start(bvb_sb[:], bvb_d[:])
            nc.gpsimd.dma_start(g_sb[:], g_d[:])
            nc.gpsimd.dma_start(ones_col[:], ones_d[0:128, 0:1].bitcast(F32R))
            nc.gpsimd.dma_start(ones_row[:], ones_d[0:1, 0:128].bitcast(F32R))

            # Q/K projections: q[d,p] = sum_c Wq[d,c] x[c,p] + bq[d]
            with (
                tc.tile_pool(name="ps_q", bufs=2, space="PSUM") as ps_q,
                tc.tile_pool(name="ps_k", bufs=2, space="PSUM") as ps_k,
                tc.tile_pool(name="ps_v", bufs=2, space="PSUM") as ps_v,
            ):
                for ci in range(NCH):
                    sl = bass.ts(ci, PCH)
                    qp = ps_q.tile([CQ, PCH], F32)
                    nc.tensor.matmul(qp[:], wqt0[:], x0[:, sl], start=True, stop=False)
                    nc.tensor.matmul(qp[:], wqt1[:], x1[:, sl], start=False, stop=True)
                    nc.scalar.activation(qsb[0:CQ, sl], qp[:], AF.Identity, bias=bq_sb[:, 0:1])
                    kp = ps_k.tile([CQ, PCH], F32)
                    nc.tensor.matmul(kp[:], wkt0[:], x0[:, sl], start=True, stop=False)
                    nc.tensor.matmul(kp[:], wkt1[:], x1[:, sl], start=False, stop=True)
                    nc.scalar.activation(ksb[0:CQ, sl], kp[:], AF.Identity, bias=bk_sb[:, 0:1])

                nc.gpsimd.dma_start(qsb[CQ:128, :], qsb[0:CQ, :])
                nc.gpsimd.dma_start(ksb[CQ:128, :], ksb[0:CQ, :])

                # V transposed: vt[q, c] = sum_c' x[c', q] WvT[c', c] + bv[c]
                for qt in range(NQT):
                    qsl = bass.ts(qt, 128)
                    vp = ps_v.tile([128, C], F32)
                    nc.tensor.matmul(vp[:], x0[:, qsl], wvt0[:], start=True, stop=False)
                    nc.tensor.matmul(vp[:], x1[:, qsl], wvt1[:], start=False, stop=True)
                    nc.vector.tensor_add(vt[:, qt * C:(qt + 1) * C], vp[:], bvb_sb[:])

            # attention main loop: ci chunks in pairs so St/O matmuls reuse
            # the loaded stationary (ldweights=False on the second use)
            with (
                tc.tile_pool(name="ps_st", bufs=1, space="PSUM") as ps_st,
                tc.tile_pool(name="ps_o", bufs=1, space="PSUM") as ps_o,
                tc.tile_pool(name="ps_dn", bufs=1, space="PSUM") as ps_dn,
                tc.tile_pool(name="ps_bc", bufs=1, space="PSUM") as ps_bc,
            ):
                for cp in range(NCH // 2):
                    psl0 = bass.ts(2 * cp, PCH)
                    psl1 = bass.ts(2 * cp + 1, PCH)
                    oc00 = ps_o.tile([128, PCH], F32)
                    oc01 = ps_o.tile([128, PCH], F32)
                    oc10 = ps_o.tile([128, PCH], F32)
                    oc11 = ps_o.tile([128, PCH], F32)
                    eprev = None
                    for qt in range(NQT + 1):
                        if qt < NQT:
                            qsl = bass.ts(qt, 128)
                            st0 = ps_st.tile([128, PCH], F32)
                            st1 = ps_st.tile([128, PCH], F32)
                            nc.tensor.matmul(st0[:], ksb[0:CQ, qsl],
                                             qsb[0:CQ, psl0],
                                             start=True, stop=True)
                            nc.tensor.matmul(st1[:], ksb[CQ:128, qsl],
                                             qsb[CQ:128, psl1],
                                             start=True, stop=True)
                        if qt > 0:
                            p = qt - 1
                            first, last = p == 0, p == NQT - 1
                            v0 = vt[:, p * C:p * C + 128]
                            v1 = vt[:, p * C + 128:(p + 1) * C]
                            e0p, e1p = eprev
                            nc.tensor.matmul(oc00[:], v0, e0p[:],
                                             start=first, stop=last)
                            h = nc.tensor.matmul(oc01[:], v0, e1p[:],
                                                 start=first, stop=last)
                            h.ins.ldweights = False
                            nc.tensor.matmul(oc10[:], v1, e0p[:],
                                             start=first, stop=last)
                            h = nc.tensor.matmul(oc11[:], v1, e1p[:],
                                                 start=first, stop=last)
                            h.ins.ldweights = False
                        if qt < NQT:
                            e0 = ework.tile([128, PCH], BF16)
                            e1 = ework.tile([128, PCH], BF16)
                            nc.scalar.activation(e0[:], st0[:], AF.Exp)
                            nc.scalar.activation(e1[:], st1[:], AF.Exp)
                            if qt == 0:
                                nc.vector.tensor_copy(esum0[:], e0[:])
                                nc.vector.tensor_copy(esum1[:], e1[:])
                            else:
                                nc.vector.tensor_add(
                                    esum0[:], esum0[:].bitcast(F32), e0[:])
                                nc.vector.tensor_add(
                                    esum1[:], esum1[:].bitcast(F32), e1[:])
                            eprev = (e0, e1)

                    for j, (esum, psl, oc0, oc1) in enumerate(
                            ((esum0, psl0, oc00, oc10), (esum1, psl1, oc01, oc11))):
                        dn = ps_dn.tile([1, PCH], F32)
                        nc.tensor.matmul(dn[:], ones_col[:], esum[:],
                                         start=True, stop=True)
                        dr = small.tile([1, PCH], F32)
                        nc.vector.reciprocal(dr[:], dn[:])
                        gd = small.tile([1, PCH], F32R)
                        nc.vector.tensor_scalar_mul(gd[:], dr[:], g_sb[0:1, 0:1])
                        bc = ps_bc.tile([128, PCH], F32)
                        nc.tensor.matmul(bc[:], ones_row[:], gd[:],
                                         start=True, stop=True)
                        bcs = small.tile([128, PCH], F32)
                        nc.vector.tensor_copy(bcs[:], bc[:])
                        for ct, (oc, xs) in enumerate(((oc0, x0), (oc1, x1))):
                            tm = outp.tile([128, PCH], F32)
                            nc.vector.tensor_mul(tm[:], oc[:], bcs[:])
                            to = outp.tile([128, PCH], F32)
                            nc.vector.tensor_add(to[:], tm[:], xs[:, psl].bitcast(F32))
                            nc.gpsimd.dma_start(out_d[ct * 128:(ct + 1) * 128, psl],
                                                to[:])
    orig_pass = nc.move_matmul_waits_to_ldweights

    def patched():
        orig_pass()
        _dedup_ldweights(nc)

    nc.move_matmul_waits_to_ldweights = patched
    nc.compile()
    return nc


_NC_CACHE = None


def kernel(x, Wq, bq, Wk, bk, Wv, bv, gamma):
    global _NC_CACHE
    if _NC_CACHE is None:
        _NC_CACHE = build_bass()
    nc = _NC_CACHE

    x = np.ascontiguousarray(x, dtype=np.float32)
    wqt = np.ascontiguousarray(np.asarray(Wq, dtype=np.float32).T)
    wkt = np.ascontiguousarray(np.asarray(Wk, dtype=np.float32).T)
    wvt = np.ascontiguousarray(np.asarray(Wv, dtype=np.float32).T)
    bq2 = np.ascontiguousarray(np.asarray(bq, dtype=np.float32).reshape(CQ, 1))
    bk2 = np.ascontiguousarray(np.asarray(bk, dtype=np.float32).reshape(CQ, 1))
    bvb = np.ascontiguousarray(
        np.broadcast_to(np.asarray(bv, dtype=np.float32)[None, :], (128, C)))
    g2 = np.ascontiguousarray(np.asarray(gamma, dtype=np.float32).reshape(1, 1))

    ones = np.ones((128, 128), dtype=np.float32)
    xf = x.reshape(N, C, P)
    in_maps = [
        {"x": xf[i], "wqt": wqt, "wkt": wkt, "wvt": wvt,
         "bq": bq2, "bk": bk2, "bvb": bvb, "ones": ones, "gamma": g2}
        for i in range(N)
    ]
    res = run_bass_kernel_spmd(nc, in_maps, list(range(N)))
    out = np.stack([res.results[i]["out"] for i in range(N)])
    return out.reshape(N, C, 64, 64).astype(np.float32, copy=False)


# revision 30
# speedup vs baseline: 1.1966x; 1.1966x over previous
import sys

sys.path.insert(0, '/opt/trn_rl_repo')

import numpy as np

import concourse.bacc as bacc
import concourse.bass as bass
import concourse.mybir as mybir
import concourse.tile as tile
from concourse.bass_utils import run_bass_kernel_spmd

N = 8
C = 256
CQ = 64
P = 4096
PCH = 512
NCH = P // PCH      # 8 p-chunks
NQT = P // 128      # 32 q-tiles
F32 = mybir.dt.float32
F32R = mybir.dt.float32r
BF16 = mybir.dt.bfloat16


def _dedup_ldweights(nc):
    # drop InstLdweights that reload the PE array with the weights it already
    # holds; their waits/updates move onto the following matmul (legalized by
    # the subsequent generate_event_semaphores pass)
    PE = mybir.EngineType.PE
    for blkw in nc.bb_map.values():
        insts = blkw.bb.instructions
        cur = None
        drop = []
        for idx, i in enumerate(insts):
            if getattr(i, 'engine', None) != PE:
                continue
            tn = type(i).__name__
            if tn == 'InstLdweights':
                key = (str(i.ins[0]), str(getattr(i, 'tile_position', None)))
                if cur is not None and key == cur:
                    drop.append(idx)
                else:
                    cur = key
            elif tn == 'InstMatmult':
                if i.ldweights is not False:
                    cur = (str(i.ins[1]),
                           str(getattr(i, 'tile_position', None)))
            elif tn == 'InstEventSemaphore':
                pass
            else:
                cur = None
        for idx in reversed(drop):
            ld = insts[idx]
            mm = None
            for j in range(idx + 1, len(insts)):
                if type(insts[j]).__name__ == 'InstMatmult':
                    mm = insts[j]
                    break
            assert mm is not None
            si = ld.sync_info
            if si is not None:
                msi = mm.sync_info
                if msi is None:
                    mm.sync_info = si
                else:
                    msi.on_wait.extend(si.on_wait)
                    msi.on_update.extend(si.on_update)
            del insts[idx]
        if drop:
            left = sum(1 for i in insts
                       if type(i).__name__ == 'InstLdweights')
            assert left + len(drop) >= len(drop)


def build_bass():
    nc = bacc.Bacc(None, target_bir_lowering=False)
    x_d = nc.dram_tensor("x", [C, P], F32, kind="ExternalInput")
    wqt_d = nc.dram_tensor("wqt", [C, CQ], F32, kind="ExternalInput")
    wkt_d = nc.dram_tensor("wkt", [C, CQ], F32, kind="ExternalInput")
    wvt_d = nc.dram_tensor("wvt", [C, C], F32, kind="ExternalInput")
    bq_d = nc.dram_tensor("bq", [CQ, 1], F32, kind="ExternalInput")
    bk_d = nc.dram_tensor("bk", [CQ, 1], F32, kind="ExternalInput")
    bvb_d = nc.dram_tensor("bvb", [128, C], F32, kind="ExternalInput")
    ones_d = nc.dram_tensor("ones", [128, 128], F32, kind="ExternalInput")
    g_d = nc.dram_tensor("gamma", [1, 1], F32, kind="ExternalInput")
    out_d = nc.dram_tensor("out", [C, P], F32, kind="ExternalOutput")

    AF = mybir.ActivationFunctionType

    with tile.TileContext(nc) as tc:
        with (
            tc.tile_pool(name="big", bufs=1) as big,
            tc.tile_pool(name="ework", bufs=3) as ework,
            tc.tile_pool(name="small", bufs=2) as small,
            tc.tile_pool(name="outp", bufs=2) as outp,
        ):
            x0 = big.tile([128, P], F32R)
            x1 = big.tile([128, P], F32R)
            qsb = big.tile([CQ, P], BF16)
            ksb = big.tile([CQ, P], BF16)
            vt = big.tile([128, NQT * C], BF16)
            esum0 = big.tile([128, PCH], F32R)
            esum1 = big.tile([128, PCH], F32R)
            wqt0 = big.tile([128, CQ], F32R)
            wqt1 = big.tile([128, CQ], F32R)
            wkt0 = big.tile([128, CQ], F32R)
            wkt1 = big.tile([128, CQ], F32R)
            wvt0 = big.tile([128, C], F32R)
            wvt1 = big.tile([128, C], F32R)
            bq_sb = big.tile([CQ, 1], F32)
            bk_sb = big.tile([CQ, 1], F32)
            bvb_sb = big.tile([128, C], F32)
            g_sb = big.tile([1, 1], F32)
            ones_col = big.tile([128, 1], F32R)
            ones_row = big.tile([1, 128], F32R)

            nc.gpsimd.dma_start(x0[:], x_d[0:128, :].bitcast(F32R))
            nc.gpsimd.dma_start(x1[:], x_d[128:256, :].bitcast(F32R))
            nc.gpsimd.dma_start(wqt0[:], wqt_d[0:128, :].bitcast(F32R))
            nc.gpsimd.dma_start(wqt1[:], wqt_d[128:256, :].bitcast(F32R))
            nc.gpsimd.dma_start(wkt0[:], wkt_d[0:128, :].bitcast(F32R))
            nc.gpsimd.dma_start(wkt1[:], wkt_d[128:256, :].bitcast(F32R))
            nc.gpsimd.dma_start(wvt0[:], wvt_d[0:128, :].bitcast(F32R))
            nc.gpsimd.dma_start(wvt1[:], wvt_d[128:256, :].bitcast(F32R))
            nc.gpsimd.dma_start(bq_sb[:], bq_d[:])
            nc.gpsimd.dma_start(bk_sb[:], bk_d[:])
            nc.gpsimd.dma_start(bvb_sb[:], bvb_d[:])
            nc.gpsimd.dma_start(g_sb[:], g_d[:])
            nc.gpsimd.dma_start(ones_col[:], ones_d[0:128, 0:1].bitcast(F32R))
            nc.gpsimd.dma_start(ones_row[:], ones_d[0:1, 0:128].bitcast(F32R))

            # Q/K projections: q[d,p] = sum_c Wq[d,c] x[c,p] + bq[d]
            with (
                tc.tile_pool(name="ps_q", bufs=2, space="PSUM") as ps_q,
                tc.tile_pool(name="ps_k", bufs=2, space="PSUM") as ps_k,
                tc.tile_pool(name="ps_v", bufs=2, space="PSUM") as ps_v,
            ):
                for ci in range(NCH):
                    sl = bass.ts(ci, PCH)
                    qp = ps_q.tile([CQ, PCH], F32)
                    nc.tensor.matmul(qp[:], wqt0[:], x0[:, sl], start=True, stop=False)
                    nc.tensor.matmul(qp[:], wqt1[:], x1[:, sl], start=False, stop=True)
                    nc.scalar.activation(qsb[:, sl], qp[:], AF.Identity, bias=bq_sb[:, 0:1])
                    kp = ps_k.tile([CQ, PCH], F32)
                    nc.tensor.matmul(kp[:], wkt0[:], x0[:, sl], start=True, stop=False)
                    nc.tensor.matmul(kp[:], wkt1[:], x1[:, sl], start=False, stop=True)
                    nc.scalar.activation(ksb[:, sl], kp[:], AF.Identity, bias=bk_sb[:, 0:1])

                # V transposed: vt[q, c] = sum_c' x[c', q] WvT[c', c] + bv[c]
                for qt in range(NQT):
                    qsl = bass.ts(qt, 128)
                    vp = ps_v.tile([128, C], F32)
                    nc.tensor.matmul(vp[:], x0[:, qsl], wvt0[:], start=True, stop=False)
                    nc.tensor.matmul(vp[:], x1[:, qsl], wvt1[:], start=False, stop=True)
                    nc.vector.tensor_add(vt[:, qt * C:(qt + 1) * C], vp[:], bvb_sb[:])

            # attention main loop: ci chunks in pairs so St/O matmuls reuse
            # the loaded stationary (ldweights=False on the second use)
            with (
                tc.tile_pool(name="ps_st", bufs=1, space="PSUM") as ps_st,
                tc.tile_pool(name="ps_o", bufs=1, space="PSUM") as ps_o,
                tc.tile_pool(name="ps_dn", bufs=1, space="PSUM") as ps_dn,
                tc.tile_pool(name="ps_bc", bufs=1, space="PSUM") as ps_bc,
            ):
                for cp in range(NCH // 2):
                    psl0 = bass.ts(2 * cp, PCH)
                    psl1 = bass.ts(2 * cp + 1, PCH)
                    oc00 = ps_o.tile([128, PCH], F32)
                    oc01 = ps_o.tile([128, PCH], F32)
                    oc10 = ps_o.tile([128, PCH], F32)
                    oc11 = ps_o.tile([128, PCH], F32)
                    eprev = None
                    for qt in range(NQT + 1):
                        if qt < NQT:
                            qsl = bass.ts(qt, 128)
                            st0 = ps_st.tile([128, PCH], F32)
                            st1 = ps_st.tile([128, PCH], F32)
                            nc.tensor.matmul(st0[:], ksb[:, qsl], qsb[:, psl0],
                                             start=True, stop=True)
                            h = nc.tensor.matmul(st1[:], ksb[:, qsl],
                                                 qsb[:, psl1],
                                                 start=True, stop=True)
                            h.ins.ldweights = False
                        if qt > 0:
                            p = qt - 1
                            first, last = p == 0, p == NQT - 1
                            v0 = vt[:, p * C:p * C + 128]
                            v1 = vt[:, p * C + 128:(p + 1) * C]
                            e0p, e1p = eprev
                            nc.tensor.matmul(oc00[:], v0, e0p[:],
                                             start=first, stop=last)
                            h = nc.tensor.matmul(oc01[:], v0, e1p[:],
                                                 start=first, stop=last)
                            h.ins.ldweights = False
                            nc.tensor.matmul(oc10[:], v1, e0p[:],
                                             start=first, stop=last)
                            h = nc.tensor.matmul(oc11[:], v1, e1p[:],
                                                 start=first, stop=last)
                            h.ins.ldweights = False
                        if qt < NQT:
                            e0 = ework.tile([128, PCH], BF16)
                            e1 = ework.tile([128, PCH], BF16)
                            nc.scalar.activation(e0[:], st0[:], AF.Exp)
                            nc.scalar.activation(e1[:], st1[:], AF.Exp)
                            if qt == 0:
                                nc.vector.tensor_copy(esum0[:], e0[:])
                                nc.vector.tensor_copy(esum1[:], e1[:])
                            else:
                                nc.vector.tensor_add(
                                    esum0[:], esum0[:].bitcast(F32), e0[:])
                                nc.vector.tensor_add(
                                    esum1[:], esum1[:].bitcast(F32), e1[:])
                            eprev = (e0, e1)

                    for j, (esum, psl, oc0, oc1) in enumerate(
                            ((esum0, psl0, oc00, oc10), (esum1, psl1, oc01, oc11))):
                        dn = ps_dn.tile([1, PCH], F32)
                        nc.tensor.matmul(dn[:], ones_col[:], esum[:],
                                         start=True, stop=True)
                        dr = small.tile([1, PCH], F32)
                        nc.vector.reciprocal(dr[:], dn[:])
                        gd = small.tile([1, PCH], F32R)
                        nc.vector.tensor_scalar_mul(gd[:], dr[:], g_sb[0:1, 0:1])
                        bc = ps_bc.tile([128, PCH], F32)
                        nc.tensor.matmul(bc[:], ones_row[:], gd[:],
                                         start=True, stop=True)
                        bcs = small.tile([128, PCH], F32)
                        nc.vector.tensor_copy(bcs[:], bc[:])
                        for ct, (oc, xs) in enumerate(((oc0, x0), (oc1, x1))):
                            tm = outp.tile([128, PCH], F32)
                            nc.vector.tensor_mul(tm[:], oc[:], bcs[:])
                            to = outp.tile([128, PCH], F32)
                            nc.vector.tensor_add(to[:], tm[:], xs[:, psl].bitcast(F32))
                            nc.gpsimd.dma_start(out_d[ct * 128:(ct + 1) * 128, psl],
                                                to[:])
    orig_pass = nc.move_matmul_waits_to_ldweights

    def patched():
        orig_pass()
        _dedup_ldweights(nc)

    nc.move_matmul_waits_to_ldweights = patched
    nc.compile()
    return nc


_NC_CACHE = None


def kernel(x, Wq, bq, Wk, bk, Wv, bv, gamma):
    global _NC_CACHE
    if _NC_CACHE is None:
        _NC_CACHE = build_bass()
    nc = _NC_CACHE

    x = np.ascontiguousarray(x, dtype=np.float32)
    wqt = np.ascontiguousarray(np.asarray(Wq, dtype=np.float32).T)
    wkt = np.ascontiguousarray(np.asarray(Wk, dtype=np.float32).T)
    wvt = np.ascontiguousarray(np.asarray(Wv, dtype=np.float32).T)
    bq2 = np.ascontiguousarray(np.asarray(bq, dtype=np.float32).reshape(CQ, 1))
    bk2 = np.ascontiguousarray(np.asarray(bk, dtype=np.float32).reshape(CQ, 1))
    bvb = np.ascontiguousarray(
        np.broadcast_to(np.asarray(bv, dtype=np.float32)[None, :], (128, C)))
    g2 = np.ascontiguousarray(np.asarray(gamma, dtype=np.float32).reshape(1, 1))

    ones = np.ones((128, 128), dtype=np.float32)
    xf = x.reshape(N, C, P)
    in_maps = [
        {"x": xf[i], "wqt": wqt, "wkt": wkt, "wvt": wvt,
         "bq": bq2, "bk": bk2, "bvb": bvb, "ones": ones, "gamma": g2}
        for i in range(N)
    ]
    res = run_bass_kernel_spmd(nc, in_maps, list(range(N)))
    out = np.stack([res.results[i]["out"] for i in range(N)])
    return out.reshape(N, C, 64, 64).astype(np.float32, copy=False)
